# revision 25
# baseline (speedup 1.0000x reference)
"""CondConv2d (MoE-routed per-sample conv) Trainium2 Bass kernel.

Data-parallel over batch: 16 samples -> 8 cores x 2 samples. Each core:
  1. routing: global-avg-pool(x) @ routing_w.T + b -> sigmoid -> r[2,8]
     (pooled sums come free as activation accum_out side-outputs)
  2. expert-weight aggregation ON THE PE: with the expert bank reshaped
     host-side to [16, 128=(e,o16), 2304=(i,kh,kw)], matmul
     bank_j.T @ S  with the "routing selector" S[(e,o'),(o,b)] =
     r[b,e]*delta(o,o') yields agg weights directly in [i, o] (lhsT)
     layout, 32 outputs per matmul (16 o-values x 2 samples).
  3. conv2d 3x3 pad 1 as 18 shifted matmuls (2 i-tiles x 9 taps): x comes
     host-pre-padded in a stride-66 flat layout so every tap is a constant
     column offset; accumulate in PSUM, extract with fused per-channel
     bias add on the scalar engine.
"""

from contextlib import ExitStack

import numpy as np

import concourse.bacc as bacc
import concourse.bass as bass
import concourse.mybir as mybir
import concourse.tile as tile
from concourse.bass_utils import run_bass_kernel_spmd

# ----- problem constants (hardcoded; kernel.py must be self-contained) -----
B, CIN, H, W = 16, 256, 64, 64
E, COUT, KK = 8, 256, 3
NCORES = 8
B_LOC = B // NCORES          # 2 samples per core
NT = CIN // 128              # 2 partition tiles for i and o
WP = W + 2                   # 66: padded row stride
HROWS = H + 4                # 68 rows: halo + 64 + halo + overflow pad
CHUNK_ROWS = 7               # output rows per PSUM chunk (7*66=462 <= 512)
NCOL_FULL = CHUNK_ROWS * WP  # 462
NCHUNKS = 10                 # 9 full chunks (63 rows) + 1 chunk (1 row)
OBLK = 16                    # o-values per aggregation j-block (128/E)
NJ = COUT // OBLK            # 16 j-blocks
TAPS = KK * KK               # 9
FP = mybir.dt.float32
BF = mybir.dt.bfloat16
BF_NP = mybir.dt.np(BF)

_OFFS = [dh * WP + dw for dh in range(3) for dw in range(3)]


def _make_pools(ctx: ExitStack, tc: "tile.TileContext"):
    return {
        "const": ctx.enter_context(tc.tile_pool(name="const", bufs=1)),
        "xpad": ctx.enter_context(tc.tile_pool(name="xpad", bufs=1)),
        "scratch": ctx.enter_context(tc.tile_pool(name="scratch", bufs=1)),
        "bank": ctx.enter_context(tc.tile_pool(name="bank", bufs=4)),
        "aggt": ctx.enter_context(tc.tile_pool(name="aggt", bufs=1)),
        "osb": ctx.enter_context(tc.tile_pool(name="osb", bufs=6)),
        "small": ctx.enter_context(tc.tile_pool(name="small", bufs=1)),
        "psag": ctx.enter_context(tc.tile_pool(name="psag", bufs=2, space="PSUM")),
        "pscv": ctx.enter_context(tc.tile_pool(name="pscv", bufs=6, space="PSUM")),
    }


def _build_body(ctx: ExitStack, tc: "tile.TileContext", aps: dict, pools: dict,
                warmup: bool = True):
    nc = tc.nc
    x, wt, bias_, rwT, rb, mask, bce, out = (
        aps["x"], aps["wt"], aps["bias"], aps["rwT"], aps["rb"],
        aps["mask"], aps["bce"], aps["out"],
    )

    const_pool = pools["const"]
    xpad_pool = pools["xpad"]
    scratch_pool = pools["scratch"]
    bank_pool = pools["bank"]
    aggt_pool = pools["aggt"]
    osb_pool = pools["osb"]
    small_pool = pools["small"]
    ps_ag_pool = pools["psag"]
    ps_cv_pool = pools["pscv"]
    ps_sm_pool = pools["psag"]

    AF = mybir.ActivationFunctionType

    # --- activation-table preloads: dummy uses so LoadActFuncSet runs at t=0
    if warmup:
        warm = small_pool.tile([1, 2], FP, name="warm")
        nc.vector.memset(warm, 0.0)
        warm2 = small_pool.tile([1, 2], FP, name="warm2")
        nc.scalar.activation(warm2, warm, AF.Sigmoid)
        nc.scalar.activation(warm2, warm, AF.Identity)
        # PE HAM warmup: keep the PE busy through the DMA-bound front so the
        # clock gate is at 8/8 when real matmuls arrive (single-shot only)
        wz = scratch_pool.tile([128, 512], BF, name="wz", tag="wz")
        nc.vector.memset(wz, 0.0)
        wps = ps_cv_pool.tile([128, 512], FP, name="wps", tag="cv")
        for i in range(60):
            nc.tensor.matmul(wps, wz[:, :128], wz, start=True, stop=True)

    # --- x first (front-DMA critical path); pooled sums via accum_out,
    #     split ACT/DVE halves so neither engine paces the chain alone ---
    HALF = (HROWS * WP) // 2
    pooledT = small_pool.tile([128, NT, 2, B_LOC], FP, name="pooledT")
    scr = scratch_pool.tile([128, HALF], BF, name="scr")
    xpads = {}
    for b in range(B_LOC):
        for t in range(NT):
            xp = xpad_pool.tile([128, HROWS, WP], BF,
                                name=f"xp_{b}_{t}", tag=f"xp_{b}_{t}")
            nc.sync.dma_start(out=xp, in_=x[b, t])
            xpads[b, t] = xp
            xpf = xp.rearrange("p h w -> p (h w)")
            # halo zeros don't change the sums
            nc.scalar.activation(
                scr, xpf[:, :HALF], AF.Copy,
                accum_out=pooledT[:, t, 0, b:b + 1],
            )
            nc.vector.tensor_reduce(
                pooledT[:, t, 1, b:b + 1], xpf[:, HALF:],
                axis=mybir.AxisListType.X, op=mybir.AluOpType.add,
            )

    # --- small/const inputs to SBUF (gpsimd queue: off the SP issue path) ---
    mask_sb = const_pool.tile([128, OBLK * B_LOC], BF, name="mask_sb")
    nc.gpsimd.dma_start(out=mask_sb, in_=mask)
    bce_sb = const_pool.tile([E, 128], FP, name="bce_sb")
    nc.gpsimd.dma_start(out=bce_sb, in_=bce)
    rwT_sb = const_pool.tile([128, NT, E], FP, name="rwT_sb")
    nc.gpsimd.dma_start(out=rwT_sb, in_=rwT)
    rb_sb = const_pool.tile([E, 1], FP, name="rb_sb")
    nc.gpsimd.dma_start(out=rb_sb, in_=rb)
    bias_sb = const_pool.tile([E, COUT], FP, name="bias_sb")
    nc.gpsimd.dma_start(out=bias_sb, in_=bias_)

    # --- routing: logits -> sigmoid -> selector S and r broadcast ---
    lg_ps = ps_sm_pool.tile([E, B_LOC], FP, name="lg_ps", tag="ag")
    for t in range(NT):
        for h in range(2):
            nc.tensor.matmul(lg_ps, rwT_sb[:, t, :], pooledT[:, t, h, :],
                             start=(t == 0 and h == 0),
                             stop=(t == NT - 1 and h == 1))
    r_sb = small_pool.tile([E, B_LOC], FP, name="r_sb")
    nc.scalar.activation(r_sb, lg_ps, AF.Sigmoid, bias=rb_sb, scale=1.0 / (H * W))

    rbc_ps = ps_sm_pool.tile([128, B_LOC], FP, name="rbc_ps", tag="ag")
    nc.tensor.matmul(rbc_ps, bce_sb, r_sb, start=True, stop=True)
    r_bc = small_pool.tile([128, B_LOC], FP, name="r_bc")
    nc.scalar.copy(r_bc, rbc_ps)

    S_sb = small_pool.tile([128, OBLK * B_LOC], BF, name="S_sb")
    nc.vector.tensor_mul(
        S_sb.rearrange("p (o b) -> p o b", b=B_LOC),
        mask_sb.rearrange("p (o b) -> p o b", b=B_LOC),
        r_bc.unsqueeze(1).to_broadcast([128, OBLK, B_LOC]),
    )

    # --- aggregated per-sample bias: [o, b] per o-tile ---
    aggb_ps = ps_sm_pool.tile([128, NT, B_LOC], FP, name="aggb_ps", tag="ag")
    for ot in range(NT):
        nc.tensor.matmul(aggb_ps[:, ot, :], bias_sb[:, ot * 128:(ot + 1) * 128],
                         r_sb, start=True, stop=True)
    aggb_sb = small_pool.tile([128, NT, B_LOC], FP, name="aggb_sb")
    nc.scalar.copy(aggb_sb, aggb_ps)

    # --- PE aggregation of expert weights into conv-lhsT layout ---
    # aggt[b, ic][i, s, o] = sum_e r[b,e] * w[e, o, i0+i, s]
    aggt = {}
    for b in range(B_LOC):
        for ic in range(NT):
            aggt[b, ic] = aggt_pool.tile([128, TAPS, COUT], BF,
                                         name=f"aggt_{b}_{ic}", tag=f"aggt_{b}_{ic}")

    def emit_agg(ot):
        for j in range(ot * (NJ // NT), (ot + 1) * (NJ // NT)):
            bank_sb = bank_pool.tile([128, NT * 128 * TAPS], BF,
                                     name=f"bank_{j}", tag="bank")
            nc.sync.dma_start(out=bank_sb, in_=wt[j])
            bank3 = bank_sb.rearrange("k (i s) -> k i s", s=TAPS)
            for ic in range(NT):
                ag_ps = ps_ag_pool.tile([128, TAPS, OBLK * B_LOC], FP,
                                        name=f"agps_{j}_{ic}", tag="ag")
                for s in range(TAPS):
                    nc.tensor.matmul(
                        ag_ps[:, s, :],
                        bank3[:, ic * 128:(ic + 1) * 128, s],
                        S_sb, start=True, stop=True,
                    )
                ag4 = ag_ps.rearrange("p s (o b) -> p s o b", b=B_LOC)
                for b in range(B_LOC):
                    nc.scalar.activation(
                        aggt[b, ic][:, :, j * OBLK:(j + 1) * OBLK],
                        ag4[:, :, :, b], AF.Copy,
                    )

    # --- conv: 18 accumulating matmuls per output chunk ---
    def emit_conv(ot):
        for b in range(B_LOC):
            xf = (xpads[b, 0].rearrange("p h w -> p (h w)"),
                  xpads[b, 1].rearrange("p h w -> p (h w)"))
            groups = [[0, 1, 2, 3], [4, 5, 6, 7], [8], [9]]
            for grp in groups:
                pss = [ps_cv_pool.tile([128, NCOL_FULL], FP,
                                       name=f"cps_{b}_{ot}_{c}", tag="cv")
                       for c in grp]
                ki = 0
                for ic in range(NT):
                    for s in range(TAPS):
                        lhsT = aggt[b, ic][:, s, ot * 128:(ot + 1) * 128]
                        for c, ps in zip(grp, pss):
                            q0 = c * NCOL_FULL
                            ncol = NCOL_FULL if c < 9 else WP
                            nc.tensor.matmul(
                                ps[:, :ncol], lhsT,
                                xf[ic][:, q0 + _OFFS[s]: q0 + _OFFS[s] + ncol],
                                start=(ki == 0), stop=(ki == NT * TAPS - 1),
                            )
                        ki += 1
                for c, ps in zip(grp, pss):
                    nrow = CHUNK_ROWS if c < 9 else 1
                    osb = osb_pool.tile([128, CHUNK_ROWS, W], BF,
                                        name=f"osb_{b}_{ot}_{c}", tag="osb")
                    nc.scalar.activation(
                        osb[:, :nrow, :],
                        ps.rearrange("p (r w) -> p r w", w=WP)[:, :nrow, :W],
                        AF.Identity, bias=aggb_sb[:, ot, b:b + 1],
                    )
                    nc.sync.dma_start(
                        out=out[b, ot * 128:(ot + 1) * 128,
                                c * CHUNK_ROWS:c * CHUNK_ROWS + nrow, :],
                        in_=osb[:, :nrow, :],
                    )

    for ot in range(NT):
        emit_agg(ot)
        emit_conv(ot)


def build_nc(reps=1):
    nc = bacc.Bacc("TRN2", debug=False)
    aps = {}
    aps["x"] = nc.declare_dram_parameter("x", [B_LOC, NT, 128, HROWS, WP], BF, isOutput=False).ap()
    aps["wt"] = nc.declare_dram_parameter("wt", [NJ, 128, NT * 128 * TAPS], BF, isOutput=False).ap()
    aps["bias"] = nc.declare_dram_parameter("bias", [E, COUT], FP, isOutput=False).ap()
    aps["rwT"] = nc.declare_dram_parameter("rwT", [128, NT, E], FP, isOutput=False).ap()
    aps["rb"] = nc.declare_dram_parameter("rb", [E, 1], FP, isOutput=False).ap()
    aps["mask"] = nc.declare_dram_parameter("mask", [128, OBLK * B_LOC], BF, isOutput=False).ap()
    aps["bce"] = nc.declare_dram_parameter("bce", [E, 128], FP, isOutput=False).ap()
    aps["out"] = nc.declare_dram_parameter("out", [B_LOC, COUT, H, W], BF, isOutput=True).ap()
    with tile.TileContext(nc) as tc, ExitStack() as ctx:
        pools = _make_pools(ctx, tc)
        for rep in range(reps):
            _build_body(ctx, tc, aps, pools, warmup=(rep == 0))
    nc.compile()
    return nc


def prep_in_maps(x, weight, bias, routing_w, routing_b):
    x = np.asarray(x, np.float32)
    weight = np.asarray(weight, np.float32)
    bias = np.asarray(bias, np.float32)
    routing_w = np.asarray(routing_w, np.float32)
    routing_b = np.asarray(routing_b, np.float32)

    # x -> bf16, zero-padded into the stride-66 conv layout
    xp = np.zeros((B, NT, 128, HROWS, WP), BF_NP)
    xp[:, :, :, 1:1 + H, 1:1 + W] = (
        x.reshape(B, NT, 128, H, W).astype(BF_NP)
    )
    # bank -> bf16 [j, (e, o16), (i, kh, kw)]
    wt = np.ascontiguousarray(
        weight.reshape(E, NJ, OBLK, CIN * KK * KK).transpose(1, 0, 2, 3)
        .reshape(NJ, 128, CIN * KK * KK)
    ).astype(BF_NP)
    # routing_w.T -> [i mod 128, i_tile, e]
    rwT = np.ascontiguousarray(
        routing_w.T.reshape(NT, 128, E).transpose(1, 0, 2)
    )
    rb = np.ascontiguousarray(routing_b.reshape(E, 1))
    # selector mask[(e,o_rel), (o', b)] = 1 iff o_rel == o'
    p_orel = np.arange(128) % OBLK
    m_o = np.arange(OBLK * B_LOC) // B_LOC
    mask = (p_orel[:, None] == m_o[None, :]).astype(BF_NP)
    # broadcast matrix bce[e, p] = 1 iff p // 16 == e
    bce = (np.arange(E)[:, None] == (np.arange(128) // OBLK)[None, :]).astype(np.float32)

    in_maps = []
    for c in range(NCORES):
        in_maps.append({
            "x": np.ascontiguousarray(xp[c * B_LOC:(c + 1) * B_LOC]),
            "wt": wt,
            "bias": bias,
            "rwT": rwT,
            "rb": rb,
            "mask": mask,
            "bce": bce,
        })
    return in_maps


_NC = None


def kernel(x, weight, bias, routing_w, routing_b):
    global _NC
    if _NC is None:
        _NC = build_nc()
    in_maps = prep_in_maps(x, weight, bias, routing_w, routing_b)
    res = run_bass_kernel_spmd(_NC, in_maps, list(range(NCORES))).results
    return np.concatenate(
        [res[c]["out"] for c in range(NCORES)], axis=0
    ).astype(np.float32)


# revision 35
# speedup vs baseline: 1.0378x; 1.0378x over previous
"""CondConv2d (MoE-routed per-sample conv) Trainium2 Bass kernel.

Data-parallel over batch: 16 samples -> 8 cores x 2 samples. Each core:
  1. routing: global-avg-pool(x) @ routing_w.T + b -> sigmoid -> r[2,8]
     (pooled sums come free as activation accum_out side-outputs)
  2. expert-weight aggregation ON THE PE: with the expert bank reshaped
     host-side to [16, 128=(e,o16), 2304=(i,kh,kw)], matmul
     bank_j.T @ S  with the "routing selector" S[(e,o'),(o,b)] =
     r[b,e]*delta(o,o') yields agg weights directly in [i, o] (lhsT)
     layout, 32 outputs per matmul (16 o-values x 2 samples).
  3. conv2d 3x3 pad 1 as 18 shifted matmuls (2 i-tiles x 9 taps): x comes
     host-pre-padded in a stride-66 flat layout so every tap is a constant
     column offset; accumulate in PSUM, extract with fused per-channel
     bias add on the scalar engine.
"""

from contextlib import ExitStack

import numpy as np

import concourse.bacc as bacc
import concourse.bass as bass
import concourse.mybir as mybir
import concourse.tile as tile
from concourse.bass_utils import run_bass_kernel_spmd

# ----- problem constants (hardcoded; kernel.py must be self-contained) -----
B, CIN, H, W = 16, 256, 64, 64
E, COUT, KK = 8, 256, 3
NCORES = 8
B_LOC = B // NCORES          # 2 samples per core
NT = CIN // 128              # 2 partition tiles for i and o
WP = W + 2                   # 66: padded row stride
HROWS = H + 4                # 68 rows: halo + 64 + halo + overflow pad
CHUNK_ROWS = 7               # output rows per PSUM chunk (7*66=462 <= 512)
NCOL_FULL = CHUNK_ROWS * WP  # 462
NCHUNKS = 10                 # 9 full chunks (63 rows) + 1 chunk (1 row)
OBLK = 16                    # o-values per aggregation j-block (128/E)
NJ = COUT // OBLK            # 16 j-blocks
TAPS = KK * KK               # 9
FP = mybir.dt.float32
BF = mybir.dt.bfloat16
BF_NP = mybir.dt.np(BF)

_OFFS = [dh * WP + dw for dh in range(3) for dw in range(3)]


def _make_pools(ctx: ExitStack, tc: "tile.TileContext"):
    return {
        "const": ctx.enter_context(tc.tile_pool(name="const", bufs=1)),
        "xpad": ctx.enter_context(tc.tile_pool(name="xpad", bufs=1)),
        "scratch": ctx.enter_context(tc.tile_pool(name="scratch", bufs=1)),
        "bank": ctx.enter_context(tc.tile_pool(name="bank", bufs=8)),
        "aggt": ctx.enter_context(tc.tile_pool(name="aggt", bufs=1)),
        "osb": ctx.enter_context(tc.tile_pool(name="osb", bufs=6)),
        "small": ctx.enter_context(tc.tile_pool(name="small", bufs=1)),
        "psag": ctx.enter_context(tc.tile_pool(name="psag", bufs=2, space="PSUM")),
        "pscv": ctx.enter_context(tc.tile_pool(name="pscv", bufs=6, space="PSUM")),
    }


def _build_body(ctx: ExitStack, tc: "tile.TileContext", aps: dict, pools: dict,
                warmup: bool = True):
    nc = tc.nc
    x, wt, bias_, rwT, rb, mask, bce, out = (
        aps["x"], aps["wt"], aps["bias"], aps["rwT"], aps["rb"],
        aps["mask"], aps["bce"], aps["out"],
    )

    const_pool = pools["const"]
    xpad_pool = pools["xpad"]
    scratch_pool = pools["scratch"]
    bank_pool = pools["bank"]
    aggt_pool = pools["aggt"]
    osb_pool = pools["osb"]
    small_pool = pools["small"]
    ps_ag_pool = pools["psag"]
    ps_cv_pool = pools["pscv"]
    ps_sm_pool = pools["psag"]

    AF = mybir.ActivationFunctionType

    # --- activation-table preloads: dummy uses so LoadActFuncSet runs at t=0
    if warmup:
        warm = small_pool.tile([1, 2], FP, name="warm")
        nc.vector.memset(warm, 0.0)
        warm2 = small_pool.tile([1, 2], FP, name="warm2")
        nc.scalar.activation(warm2, warm, AF.Sigmoid)
        nc.scalar.activation(warm2, warm, AF.Identity)
        # PE HAM warmup: keep the PE busy through the DMA-bound front so the
        # clock gate is at 8/8 when real matmuls arrive (single-shot only)
        wz = scratch_pool.tile([128, 512], BF, name="wz", tag="wz")
        nc.vector.memset(wz, 0.0)
        wps = ps_cv_pool.tile([128, 512], FP, name="wps", tag="cv")
        for i in range(60):
            nc.tensor.matmul(wps, wz[:, :128], wz, start=True, stop=True)

    # --- x first (front-DMA critical path); pooled sums via accum_out,
    #     split ACT/DVE halves so neither engine paces the chain alone ---
    HALF = (HROWS * WP) // 2
    pooledT = small_pool.tile([128, NT, 2, B_LOC], FP, name="pooledT")
    scr = scratch_pool.tile([128, HALF], BF, name="scr")
    xpads = {}
    for b in range(B_LOC):
        for t in range(NT):
            xp = xpad_pool.tile([128, HROWS, WP], BF,
                                name=f"xp_{b}_{t}", tag=f"xp_{b}_{t}")
            nc.sync.dma_start(out=xp, in_=x[b, t])
            xpads[b, t] = xp
            xpf = xp.rearrange("p h w -> p (h w)")
            # halo zeros don't change the sums
            nc.scalar.activation(
                scr, xpf[:, :HALF], AF.Copy,
                accum_out=pooledT[:, t, 0, b:b + 1],
            )
            nc.vector.tensor_reduce(
                pooledT[:, t, 1, b:b + 1], xpf[:, HALF:],
                axis=mybir.AxisListType.X, op=mybir.AluOpType.add,
            )

    # --- small/const inputs to SBUF (gpsimd queue: off the SP issue path) ---
    mask_sb = const_pool.tile([128, OBLK * B_LOC], BF, name="mask_sb")
    nc.gpsimd.dma_start(out=mask_sb, in_=mask)
    bce_sb = const_pool.tile([E, 128], FP, name="bce_sb")
    nc.gpsimd.dma_start(out=bce_sb, in_=bce)
    rwT_sb = const_pool.tile([128, NT, E], FP, name="rwT_sb")
    nc.gpsimd.dma_start(out=rwT_sb, in_=rwT)
    rb_sb = const_pool.tile([E, 1], FP, name="rb_sb")
    nc.gpsimd.dma_start(out=rb_sb, in_=rb)
    bias_sb = const_pool.tile([E, COUT], FP, name="bias_sb")
    nc.gpsimd.dma_start(out=bias_sb, in_=bias_)

    # --- routing: logits -> sigmoid -> selector S and r broadcast ---
    lg_ps = ps_sm_pool.tile([E, B_LOC], FP, name="lg_ps", tag="ag")
    for t in range(NT):
        for h in range(2):
            nc.tensor.matmul(lg_ps, rwT_sb[:, t, :], pooledT[:, t, h, :],
                             start=(t == 0 and h == 0),
                             stop=(t == NT - 1 and h == 1))
    r_sb = small_pool.tile([E, B_LOC], FP, name="r_sb")
    nc.scalar.activation(r_sb, lg_ps, AF.Sigmoid, bias=rb_sb, scale=1.0 / (H * W))

    rbc_ps = ps_sm_pool.tile([128, B_LOC], FP, name="rbc_ps", tag="ag")
    nc.tensor.matmul(rbc_ps, bce_sb, r_sb, start=True, stop=True)
    r_bc = small_pool.tile([128, B_LOC], FP, name="r_bc")
    nc.scalar.copy(r_bc, rbc_ps)

    S_sb = small_pool.tile([128, OBLK * B_LOC], BF, name="S_sb")
    nc.vector.tensor_mul(
        S_sb.rearrange("p (o b) -> p o b", b=B_LOC),
        mask_sb.rearrange("p (o b) -> p o b", b=B_LOC),
        r_bc.unsqueeze(1).to_broadcast([128, OBLK, B_LOC]),
    )

    # --- aggregated per-sample bias: [o, b] per o-tile ---
    aggb_ps = ps_sm_pool.tile([128, NT, B_LOC], FP, name="aggb_ps", tag="ag")
    for ot in range(NT):
        nc.tensor.matmul(aggb_ps[:, ot, :], bias_sb[:, ot * 128:(ot + 1) * 128],
                         r_sb, start=True, stop=True)
    aggb_sb = small_pool.tile([128, NT, B_LOC], FP, name="aggb_sb")
    nc.scalar.copy(aggb_sb, aggb_ps)

    # --- PE aggregation of expert weights into conv-lhsT layout ---
    # aggt[b, ic, ot][i, s, o] = sum_e r[b,e] * w[e, ot*128+o, i0+i, s]
    # (split by ot so ot=1 extraction writes don't WAR-couple to ot=0 reads)
    aggt = {}
    for b in range(B_LOC):
        for ic in range(NT):
            for ot in range(NT):
                aggt[b, ic, ot] = aggt_pool.tile(
                    [128, TAPS, 128], BF,
                    name=f"aggt_{b}_{ic}_{ot}", tag=f"aggt_{b}_{ic}_{ot}")

    def emit_agg(ot):
        for j in range(ot * (NJ // NT), (ot + 1) * (NJ // NT)):
            bank_sb = bank_pool.tile([128, NT * 128 * TAPS], BF,
                                     name=f"bank_{j}", tag="bank")
            # gpsimd queue: keeps bank prefetch off the SP queue, which
            # head-of-line blocks behind extraction-gated out DMAs
            nc.gpsimd.dma_start(out=bank_sb, in_=wt[j])
            bank3 = bank_sb.rearrange("k (i s) -> k i s", s=TAPS)
            for ic in range(NT):
                ag_ps = ps_ag_pool.tile([128, TAPS, OBLK * B_LOC], FP,
                                        name=f"agps_{j}_{ic}", tag="ag")
                for s in range(TAPS):
                    nc.tensor.matmul(
                        ag_ps[:, s, :],
                        bank3[:, ic * 128:(ic + 1) * 128, s],
                        S_sb, start=True, stop=True,
                    )
                ag4 = ag_ps.rearrange("p s (o b) -> p s o b", b=B_LOC)
                jr = j - ot * (NJ // NT)
                for b in range(B_LOC):
                    # DVE (not ACT): keeps agg extraction off the in-order
                    # ACT queue, which is busy with conv extractions
                    nc.vector.tensor_copy(
                        aggt[b, ic, ot][:, :, jr * OBLK:(jr + 1) * OBLK],
                        ag4[:, :, :, b],
                    )

    # --- conv: 18 accumulating matmuls per output chunk ---
    def emit_conv(ot, bs):
        for b in bs:
            xf = (xpads[b, 0].rearrange("p h w -> p (h w)"),
                  xpads[b, 1].rearrange("p h w -> p (h w)"))
            groups = [[0, 1, 2, 3], [4, 5, 6, 7], [8], [9]]
            for grp in groups:
                pss = [ps_cv_pool.tile([128, NCOL_FULL], FP,
                                       name=f"cps_{b}_{ot}_{c}", tag="cv")
                       for c in grp]
                ki = 0
                for ic in range(NT):
                    for s in range(TAPS):
                        lhsT = aggt[b, ic, ot][:, s, :]
                        for c, ps in zip(grp, pss):
                            q0 = c * NCOL_FULL
                            ncol = NCOL_FULL if c < 9 else WP
                            nc.tensor.matmul(
                                ps[:, :ncol], lhsT,
                                xf[ic][:, q0 + _OFFS[s]: q0 + _OFFS[s] + ncol],
                                start=(ki == 0), stop=(ki == NT * TAPS - 1),
                            )
                        ki += 1
                for c, ps in zip(grp, pss):
                    nrow = CHUNK_ROWS if c < 9 else 1
                    osb = osb_pool.tile([128, CHUNK_ROWS, W], BF,
                                        name=f"osb_{b}_{ot}_{c}", tag="osb")
                    nc.scalar.activation(
                        osb[:, :nrow, :],
                        ps.rearrange("p (r w) -> p r w", w=WP)[:, :nrow, :W],
                        AF.Identity, bias=aggb_sb[:, ot, b:b + 1],
                    )
                    nc.sync.dma_start(
                        out=out[b, ot * 128:(ot + 1) * 128,
                                c * CHUNK_ROWS:c * CHUNK_ROWS + nrow, :],
                        in_=osb[:, :nrow, :],
                    )

    # agg(1) sits between the two ot=0 conv phases: its matmuls slot into
    # the PE stream bubble-free, and its outputs are ready well before ot=1
    emit_agg(0)
    emit_conv(0, [0])
    emit_agg(1)
    emit_conv(0, [1])
    emit_conv(1, [0, 1])


def build_nc(reps=1):
    nc = bacc.Bacc("TRN2", debug=False)
    aps = {}
    aps["x"] = nc.declare_dram_parameter("x", [B_LOC, NT, 128, HROWS, WP], BF, isOutput=False).ap()
    aps["wt"] = nc.declare_dram_parameter("wt", [NJ, 128, NT * 128 * TAPS], BF, isOutput=False).ap()
    aps["bias"] = nc.declare_dram_parameter("bias", [E, COUT], FP, isOutput=False).ap()
    aps["rwT"] = nc.declare_dram_parameter("rwT", [128, NT, E], FP, isOutput=False).ap()
    aps["rb"] = nc.declare_dram_parameter("rb", [E, 1], FP, isOutput=False).ap()
    aps["mask"] = nc.declare_dram_parameter("mask", [128, OBLK * B_LOC], BF, isOutput=False).ap()
    aps["bce"] = nc.declare_dram_parameter("bce", [E, 128], FP, isOutput=False).ap()
    aps["out"] = nc.declare_dram_parameter("out", [B_LOC, COUT, H, W], BF, isOutput=True).ap()
    with tile.TileContext(nc) as tc, ExitStack() as ctx:
        pools = _make_pools(ctx, tc)
        for rep in range(reps):
            _build_body(ctx, tc, aps, pools, warmup=(rep == 0))
    nc.compile()
    return nc


def prep_in_maps(x, weight, bias, routing_w, routing_b):
    x = np.asarray(x, np.float32)
    weight = np.asarray(weight, np.float32)
    bias = np.asarray(bias, np.float32)
    routing_w = np.asarray(routing_w, np.float32)
    routing_b = np.asarray(routing_b, np.float32)

    # x -> bf16, zero-padded into the stride-66 conv layout
    xp = np.zeros((B, NT, 128, HROWS, WP), BF_NP)
    xp[:, :, :, 1:1 + H, 1:1 + W] = (
        x.reshape(B, NT, 128, H, W).astype(BF_NP)
    )
    # bank -> bf16 [j, (e, o16), (i, kh, kw)]
    wt = np.ascontiguousarray(
        weight.reshape(E, NJ, OBLK, CIN * KK * KK).transpose(1, 0, 2, 3)
        .reshape(NJ, 128, CIN * KK * KK)
    ).astype(BF_NP)
    # routing_w.T -> [i mod 128, i_tile, e]
    rwT = np.ascontiguousarray(
        routing_w.T.reshape(NT, 128, E).transpose(1, 0, 2)
    )
    rb = np.ascontiguousarray(routing_b.reshape(E, 1))
    # selector mask[(e,o_rel), (o', b)] = 1 iff o_rel == o'
    p_orel = np.arange(128) % OBLK
    m_o = np.arange(OBLK * B_LOC) // B_LOC
    mask = (p_orel[:, None] == m_o[None, :]).astype(BF_NP)
    # broadcast matrix bce[e, p] = 1 iff p // 16 == e
    bce = (np.arange(E)[:, None] == (np.arange(128) // OBLK)[None, :]).astype(np.float32)

    in_maps = []
    for c in range(NCORES):
        in_maps.append({
            "x": np.ascontiguousarray(xp[c * B_LOC:(c + 1) * B_LOC]),
            "wt": wt,
            "bias": bias,
            "rwT": rwT,
            "rb": rb,
            "mask": mask,
            "bce": bce,
        })
    return in_maps


_NC = None


def kernel(x, weight, bias, routing_w, routing_b):
    global _NC
    if _NC is None:
        _NC = build_nc()
    in_maps = prep_in_maps(x, weight, bias, routing_w, routing_b)
    res = run_bass_kernel_spmd(_NC, in_maps, list(range(NCORES))).results
    return np.concatenate(
        [res[c]["out"] for c in range(NCORES)], axis=0
    ).astype(np.float32)
